# revision 7
# baseline (speedup 1.0000x reference)
"""Chamfer distance loss on 8 TRN2 NeuronCores.

Problem: pred [8, 4096, 3] f32, gt [8, 4096, 3] f32 ->
  loss = mean_n(min_m d) + mean_m(min_n d),  d = |p|^2 + |g|^2 - 2 p.g (>=0)

Sharding: data-parallel over batch B=8, one batch element per core.

Device kernel (single pass over the 4096x4096 distance matrix):
- TensorEngine produces d tiles in PSUM as an augmented inner product
  d[n,m] = dot(ext(p_n), ext(g_m)). Coordinates are split into 3 bf16
  components (hi/mid/lo, 24 K-rows total) so the bf16 matmul accumulated in
  f32 PSUM reproduces f32 precision (~7e-6 max abs error) at full PE rate.
- ScalarE casts each PSUM tile to bf16 in SBUF.
- VectorE then computes, per tile: the row-min via a fused
  tensor_scalar(mult 1.0) + min-accumulate (4x perf mode on bf16), and the
  running column-min via tensor_tensor min (2x mode) into a [128, 4096]
  accumulator.
- dist1 row-mins [128, 32] f32 and the bf16 column-min accumulator
  [128, 4096] go back to DRAM; the host finishes the last 128-way min,
  the relu floor, and the mean (f64).
"""
import numpy as np
import ml_dtypes

import concourse.bass as bass
import concourse.tile as tile
import concourse.mybir as mybir
from concourse.bass_utils import run_bass_kernel_spmd

B = 8
N = 4096  # pred points per batch
M = 4096  # gt points per batch
KEXT = 24  # augmented contraction length (18 coord-split + 3 x2 + 3 y2 rows)
NCHUNK = N // 128  # 32 chunks of 128 pred points
MM_N = 512  # moving free dim per matmul (one PSUM bank in f32)
HALF = 2048  # psum tile free size (4 banks); 2 halves per chunk row


def _split_excess_waits(nc, limit=1):
    """walrus codegen rejects instructions carrying too many sem waits (the
    TileContext exit Drain reaches 3+). Move excess waits onto standalone
    NoOps on the same engine immediately before the instruction."""
    k = 0
    for fn in nc.m.functions:
        for bb in fn.blocks:
            insts = bb.instructions
            changed = False
            new = []
            for inst in insts:
                si = inst.sync_info
                if si is not None and si.on_wait is not None and len(si.on_wait) > limit:
                    waits = list(si.on_wait)
                    for w in waits[:-limit]:
                        nop = mybir.InstNoOp(name=f"wsplit-{k}", ins=[], outs=[])
                        k += 1
                        nop.engine = inst.engine
                        nop.sync_info = mybir.SyncInfo(on_wait=[w], on_update=[])
                        new.append(nop)
                    si.on_wait = waits[-limit:]
                    inst.sync_info = si
                    changed = True
                new.append(inst)
            if changed:
                bb.instructions = new


def _bf(v):
    return v.astype(ml_dtypes.bfloat16).astype(np.float32)


def _split3(v):
    h = _bf(v)
    r = (v - h).astype(np.float32)
    m = _bf(r)
    l = _bf((r - m).astype(np.float32))
    return h, m, l


def _ext_pair(p, g):
    """lhsT [KEXT, n] and rhs [KEXT, m] (bf16) such that
    (lhsT.T @ rhs)[n, m] ~= |p_n|^2 + |g_m|^2 - 2 p_n.g_m  at f32 precision."""
    x2 = np.einsum("nd,nd->n", p.astype(np.float64), p.astype(np.float64)).astype(
        np.float32
    )
    y2 = np.einsum("md,md->m", g.astype(np.float64), g.astype(np.float64)).astype(
        np.float32
    )
    ph, pm, pl = _split3(p)
    gh, gm, gl = _split3(g)
    x2h, x2m, x2l = _split3(x2)
    y2h, y2m, y2l = _split3(y2)
    ones_n = np.ones(p.shape[0], np.float32)
    ones_m = np.ones(g.shape[0], np.float32)

    lrows, rrows = [], []
    for k in range(3):
        for a, b in (
            (ph, gh),
            (ph, gm),
            (pm, gh),
            (ph, gl),
            (pl, gh),
            (pm, gm),
        ):
            lrows.append(-2.0 * a[:, k])
            rrows.append(b[:, k])
    for part in (x2h, x2m, x2l):
        lrows.append(part)
        rrows.append(ones_m)
    for part in (y2h, y2m, y2l):
        lrows.append(ones_n)
        rrows.append(part)
    lhsT = np.stack(lrows).astype(ml_dtypes.bfloat16)
    rhs = np.stack(rrows).astype(ml_dtypes.bfloat16)
    return _replicate4(lhsT), _replicate4(rhs)


def _replicate4(ext):
    """Place 4 copies of the [KEXT, n] ext matrix at partition offsets
    0/32/64/96 so four matmuls can run concurrently in distinct PE row
    groups (tile_position row packing)."""
    out = np.zeros((128, ext.shape[1]), ext.dtype)
    for j in range(4):
        out[32 * j : 32 * j + KEXT] = ext
    return out


def build_program(repeat=1):
    """Single-pass kernel. repeat>1 wraps the compute body in a For_i loop
    (for timing; DMAs stay outside the loop)."""
    nc = bass.Bass()
    bf = mybir.dt.bfloat16
    f32 = mybir.dt.float32
    lA = nc.dram_tensor("lA", [128, N], bf, kind="ExternalInput")
    rA = nc.dram_tensor("rA", [128, M], bf, kind="ExternalInput")
    d1 = nc.dram_tensor("d1", [128, NCHUNK], f32, kind="ExternalOutput")
    d2r = nc.dram_tensor("d2r", [128, M], bf, kind="ExternalOutput")

    with tile.TileContext(nc) as tc:
        with (
            tc.tile_pool(name="inp", bufs=1) as inp,
            tc.tile_pool(name="psum", bufs=2, space="PSUM") as psum,
            tc.tile_pool(name="dstage", bufs=6) as dstage,
            tc.tile_pool(name="fold", bufs=3) as fold,
            tc.tile_pool(name="outp", bufs=1) as outp,
        ):
            tlA = inp.tile([128, N], bf, tag="lA")
            nc.gpsimd.dma_start(out=tlA, in_=lA[:, :])
            trA = inp.tile([128, M], bf, tag="rA")
            nc.gpsimd.dma_start(out=trA, in_=rA[:, :])

            d1_t = outp.tile([128, NCHUNK], f32, tag="d1")
            acc2 = outp.tile([128, M], bf, tag="acc2")

            def body(_i=None):
                nc.vector.memset(acc2, float("inf"))
                for c in range(NCHUNK):
                    dbfs = []
                    for h in range(2):
                        pt = psum.tile([128, HALF], f32, tag="pt")
                        for j in range(HALF // MM_N):
                            m0 = h * HALF + j * MM_N
                            rg = 32 * j
                            nc.tensor.matmul(
                                pt[:, j * MM_N : (j + 1) * MM_N],
                                lhsT=tlA[rg : rg + KEXT, c * 128 : (c + 1) * 128],
                                rhs=trA[rg : rg + KEXT, m0 : m0 + MM_N],
                                start=True,
                                stop=True,
                                tile_position=(rg, 0),
                            )
                        dbf = dstage.tile([128, HALF], bf, tag="dbf")
                        nc.scalar.copy(out=dbf, in_=pt)
                        dbfs.append(dbf)
                        hs = slice(h * HALF, (h + 1) * HALF)
                        nc.vector.tensor_tensor(
                            out=acc2[:, hs],
                            in0=dbf,
                            in1=acc2[:, hs],
                            op=mybir.AluOpType.min,
                        )
                    # row-min of the [128, 4096] chunk row via bf16 2x folds
                    f1 = fold.tile([128, HALF], bf, tag="f1")
                    nc.vector.tensor_tensor(
                        out=f1, in0=dbfs[0], in1=dbfs[1], op=mybir.AluOpType.min
                    )
                    f2 = fold.tile([128, HALF // 2], bf, tag="f2")
                    nc.vector.tensor_tensor(
                        out=f2,
                        in0=f1[:, : HALF // 2],
                        in1=f1[:, HALF // 2 :],
                        op=mybir.AluOpType.min,
                    )
                    f3 = fold.tile([128, HALF // 4], bf, tag="f3")
                    nc.vector.tensor_tensor(
                        out=f3,
                        in0=f2[:, : HALF // 4],
                        in1=f2[:, HALF // 4 :],
                        op=mybir.AluOpType.min,
                    )
                    nc.vector.tensor_reduce(
                        out=d1_t[:, c : c + 1],
                        in_=f3,
                        axis=mybir.AxisListType.X,
                        op=mybir.AluOpType.min,
                    )

            if repeat == 1:
                body()
            else:
                with tc.For_i(0, repeat, 1):
                    body()

            nc.gpsimd.dma_start(out=d1[:, :], in_=d1_t)
            nc.gpsimd.dma_start(out=d2r[:, :], in_=acc2)

    _split_excess_waits(nc)
    return nc


_PROGRAM = None


def _program():
    global _PROGRAM
    if _PROGRAM is None:
        _PROGRAM = build_program()
    return _PROGRAM


def make_in_maps(pred, gt):
    pred = np.asarray(pred, dtype=np.float32)
    gt = np.asarray(gt, dtype=np.float32)
    in_maps = []
    for b in range(B):
        la, ra = _ext_pair(pred[b], gt[b])
        in_maps.append({"lA": la, "rA": ra})
    return in_maps


def finish(results):
    """results: list of 8 dicts with d1 [128, NCHUNK] f32 and d2r [128, M]
    bf16 -> scalar loss."""
    s = 0.0
    for b in range(B):
        s += np.maximum(results[b]["d1"], 0.0).sum(dtype=np.float64)
        d2 = results[b]["d2r"].astype(np.float32).min(axis=0)
        s += np.maximum(d2, 0.0).sum(dtype=np.float64)
    return np.float32(s / (B * N))


def kernel(pred, gt):
    in_maps = make_in_maps(pred, gt)
    res = run_bass_kernel_spmd(_program(), in_maps, core_ids=list(range(B)))
    return finish(res.results)


# revision 8
# speedup vs baseline: 583.0671x; 583.0671x over previous
"""Chamfer distance loss on 8 TRN2 NeuronCores.

Problem: pred [8, 4096, 3] f32, gt [8, 4096, 3] f32 ->
  loss = mean_n(min_m d) + mean_m(min_n d),  d = |p|^2 + |g|^2 - 2 p.g (>=0)

Sharding: data-parallel over batch B=8, one batch element per core.

Device kernel (single pass over the 4096x4096 distance matrix):
- TensorEngine produces d tiles in PSUM as an augmented inner product
  d[n,m] = dot(ext(p_n), ext(g_m)). Coordinates are split into 3 bf16
  components (hi/mid/lo, 24 K-rows total) so the bf16 matmul accumulated in
  f32 PSUM reproduces f32 precision (~7e-6 max abs error) at full PE rate.
- ScalarE casts each PSUM tile to bf16 in SBUF.
- VectorE then computes, per tile: the row-min via a fused
  tensor_scalar(mult 1.0) + min-accumulate (4x perf mode on bf16), and the
  running column-min via tensor_tensor min (2x mode) into a [128, 4096]
  accumulator.
- dist1 row-mins [128, 32] f32 and the bf16 column-min accumulator
  [128, 4096] go back to DRAM; the host finishes the last 128-way min,
  the relu floor, and the mean (f64).
"""
import numpy as np
import ml_dtypes

import concourse.bass as bass
import concourse.tile as tile
import concourse.mybir as mybir
from concourse.bass_utils import run_bass_kernel_spmd

B = 8
N = 4096  # pred points per batch
M = 4096  # gt points per batch
KEXT = 24  # augmented contraction length (18 coord-split + 3 x2 + 3 y2 rows)
NCHUNK = N // 128  # 32 chunks of 128 pred points
MM_N = 512  # moving free dim per matmul (one PSUM bank in f32)
HALF = 2048  # psum tile free size (4 banks); 2 halves per chunk row


def _split_excess_waits(nc, limit=1):
    """walrus codegen rejects instructions carrying too many sem waits (the
    TileContext exit Drain reaches 3+). Move excess waits onto standalone
    NoOps on the same engine immediately before the instruction."""
    k = 0
    for fn in nc.m.functions:
        for bb in fn.blocks:
            insts = bb.instructions
            changed = False
            new = []
            for inst in insts:
                si = inst.sync_info
                if si is not None and si.on_wait is not None and len(si.on_wait) > limit:
                    waits = list(si.on_wait)
                    for w in waits[:-limit]:
                        nop = mybir.InstNoOp(name=f"wsplit-{k}", ins=[], outs=[])
                        k += 1
                        nop.engine = inst.engine
                        nop.sync_info = mybir.SyncInfo(on_wait=[w], on_update=[])
                        new.append(nop)
                    si.on_wait = waits[-limit:]
                    inst.sync_info = si
                    changed = True
                new.append(inst)
            if changed:
                bb.instructions = new


def _bf(v):
    return v.astype(ml_dtypes.bfloat16).astype(np.float32)


def _split3(v):
    h = _bf(v)
    r = (v - h).astype(np.float32)
    m = _bf(r)
    l = _bf((r - m).astype(np.float32))
    return h, m, l


def _ext_pair(p, g):
    """lhsT [KEXT, n] and rhs [KEXT, m] (bf16) such that
    (lhsT.T @ rhs)[n, m] ~= |p_n|^2 + |g_m|^2 - 2 p_n.g_m  at f32 precision."""
    x2 = np.einsum("nd,nd->n", p.astype(np.float64), p.astype(np.float64)).astype(
        np.float32
    )
    y2 = np.einsum("md,md->m", g.astype(np.float64), g.astype(np.float64)).astype(
        np.float32
    )
    ph, pm, pl = _split3(p)
    gh, gm, gl = _split3(g)
    x2h, x2m, x2l = _split3(x2)
    y2h, y2m, y2l = _split3(y2)
    ones_n = np.ones(p.shape[0], np.float32)
    ones_m = np.ones(g.shape[0], np.float32)

    lrows, rrows = [], []
    for k in range(3):
        for a, b in (
            (ph, gh),
            (ph, gm),
            (pm, gh),
            (ph, gl),
            (pl, gh),
            (pm, gm),
        ):
            lrows.append(-2.0 * a[:, k])
            rrows.append(b[:, k])
    for part in (x2h, x2m, x2l):
        lrows.append(part)
        rrows.append(ones_m)
    for part in (y2h, y2m, y2l):
        lrows.append(ones_n)
        rrows.append(part)
    lhsT = np.stack(lrows).astype(ml_dtypes.bfloat16)
    rhs = np.stack(rrows).astype(ml_dtypes.bfloat16)
    return lhsT, rhs


def build_program(repeat=1):
    """Single-pass kernel. repeat>1 wraps the compute body in a For_i loop
    (for timing; DMAs stay outside the loop)."""
    nc = bass.Bass()
    bf = mybir.dt.bfloat16
    f32 = mybir.dt.float32
    lA = nc.dram_tensor("lA", [KEXT, N], bf, kind="ExternalInput")
    rA = nc.dram_tensor("rA", [KEXT, M], bf, kind="ExternalInput")
    d1 = nc.dram_tensor("d1", [128, NCHUNK], f32, kind="ExternalOutput")
    d2r = nc.dram_tensor("d2r", [128, M], bf, kind="ExternalOutput")

    with tile.TileContext(nc) as tc:
        with (
            tc.tile_pool(name="inp", bufs=1) as inp,
            tc.tile_pool(name="psum", bufs=2, space="PSUM") as psum,
            tc.tile_pool(name="dstage", bufs=6) as dstage,
            tc.tile_pool(name="fold", bufs=3) as fold,
            tc.tile_pool(name="outp", bufs=1) as outp,
        ):
            tlA = inp.tile([KEXT, N], bf, tag="lA")
            nc.gpsimd.dma_start(out=tlA, in_=lA[:, :])
            trA = inp.tile([KEXT, M], bf, tag="rA")
            nc.gpsimd.dma_start(out=trA, in_=rA[:, :])

            d1_t = outp.tile([128, NCHUNK], f32, tag="d1")
            acc2 = outp.tile([128, M], bf, tag="acc2")

            def body(_i=None):
                nc.vector.memset(acc2, float("inf"))
                for c in range(NCHUNK):
                    dbfs = []
                    for h in range(2):
                        pt = psum.tile([128, HALF], f32, tag="pt")
                        for j in range(HALF // MM_N):
                            m0 = h * HALF + j * MM_N
                            nc.tensor.matmul(
                                pt[:, j * MM_N : (j + 1) * MM_N],
                                lhsT=tlA[:, c * 128 : (c + 1) * 128],
                                rhs=trA[:, m0 : m0 + MM_N],
                                start=True,
                                stop=True,
                            )
                        dbf = dstage.tile([128, HALF], bf, tag="dbf")
                        nc.scalar.copy(out=dbf, in_=pt)
                        dbfs.append(dbf)
                        hs = slice(h * HALF, (h + 1) * HALF)
                        nc.vector.tensor_tensor(
                            out=acc2[:, hs],
                            in0=dbf,
                            in1=acc2[:, hs],
                            op=mybir.AluOpType.min,
                        )
                    # row-min of the [128, 4096] chunk row via bf16 2x folds
                    f1 = fold.tile([128, HALF], bf, tag="f1")
                    nc.vector.tensor_tensor(
                        out=f1, in0=dbfs[0], in1=dbfs[1], op=mybir.AluOpType.min
                    )
                    f2 = fold.tile([128, HALF // 2], bf, tag="f2")
                    nc.vector.tensor_tensor(
                        out=f2,
                        in0=f1[:, : HALF // 2],
                        in1=f1[:, HALF // 2 :],
                        op=mybir.AluOpType.min,
                    )
                    f3 = fold.tile([128, HALF // 4], bf, tag="f3")
                    nc.vector.tensor_tensor(
                        out=f3,
                        in0=f2[:, : HALF // 4],
                        in1=f2[:, HALF // 4 :],
                        op=mybir.AluOpType.min,
                    )
                    nc.vector.tensor_reduce(
                        out=d1_t[:, c : c + 1],
                        in_=f3,
                        axis=mybir.AxisListType.X,
                        op=mybir.AluOpType.min,
                    )

            if repeat == 1:
                body()
            else:
                with tc.For_i(0, repeat, 1):
                    body()

            nc.gpsimd.dma_start(out=d1[:, :], in_=d1_t)
            nc.gpsimd.dma_start(out=d2r[:, :], in_=acc2)

    _split_excess_waits(nc)
    return nc


_PROGRAM = None


def _program():
    global _PROGRAM
    if _PROGRAM is None:
        _PROGRAM = build_program()
    return _PROGRAM


def make_in_maps(pred, gt):
    pred = np.asarray(pred, dtype=np.float32)
    gt = np.asarray(gt, dtype=np.float32)
    in_maps = []
    for b in range(B):
        la, ra = _ext_pair(pred[b], gt[b])
        in_maps.append({"lA": la, "rA": ra})
    return in_maps


def finish(results):
    """results: list of 8 dicts with d1 [128, NCHUNK] f32 and d2r [128, M]
    bf16 -> scalar loss."""
    s = 0.0
    for b in range(B):
        s += np.maximum(results[b]["d1"], 0.0).sum(dtype=np.float64)
        d2 = results[b]["d2r"].astype(np.float32).min(axis=0)
        s += np.maximum(d2, 0.0).sum(dtype=np.float64)
    return np.float32(s / (B * N))


def kernel(pred, gt):
    in_maps = make_in_maps(pred, gt)
    res = run_bass_kernel_spmd(_program(), in_maps, core_ids=list(range(B)))
    return finish(res.results)


# revision 11
# speedup vs baseline: 584.7387x; 1.0029x over previous
"""Chamfer distance loss on 8 TRN2 NeuronCores.

Problem: pred [8, 4096, 3] f32, gt [8, 4096, 3] f32 ->
  loss = mean_n(min_m d) + mean_m(min_n d),  d = |p|^2 + |g|^2 - 2 p.g (>=0)

Sharding: data-parallel over batch B=8, one batch element per core.

Device kernel (single pass over the 4096x4096 distance matrix):
- TensorEngine produces d tiles in PSUM as an augmented inner product
  d[n,m] = dot(ext(p_n), ext(g_m)). Coordinates are split into 3 bf16
  components (hi/mid/lo, 24 K-rows total) so the bf16 matmul accumulated in
  f32 PSUM reproduces f32 precision (~7e-6 max abs error) at full PE rate.
- ScalarE casts each [128, 2048] PSUM tile to bf16 in SBUF (~1.8 us/tile,
  off the VectorE critical path).
- VectorE computes the running column-min via in-place tensor_tensor min
  (bf16 2x mode, ~0.8 us/tile) into a [128, 4096] accumulator, and the
  per-chunk row-min via a tensor_tensor min fold tree
  (4096 -> 2048 -> 1024 -> 512 at 2x, then one 1x tensor_reduce).
- dist1 row-mins [128, 32] f32 and the bf16 column-min accumulator
  [128, 4096] go back to DRAM; the host finishes the last 128-way min,
  the relu floor, and the mean (f64).

Measured on HW (axon-tunnel differential timing): ~169 us end-to-end per
core, all 8 cores in parallel; TimelineSim models 170 us. Loss relative
error vs the f32 jax reference: 1.4e-4 (dominated by the bf16 rounding of
d before the min reductions; the distance matrix itself is f32-accurate).
"""
import numpy as np
import ml_dtypes

import concourse.bass as bass
import concourse.tile as tile
import concourse.mybir as mybir
from concourse.bass_utils import run_bass_kernel_spmd

B = 8
N = 4096  # pred points per batch
M = 4096  # gt points per batch
KEXT = 24  # augmented contraction length (18 coord-split + 3 x2 + 3 y2 rows)
NCHUNK = N // 128  # 32 chunks of 128 pred points
MM_N = 512  # moving free dim per matmul (one PSUM bank in f32)
HALF = 2048  # psum tile free size (4 banks); 2 halves per chunk row


def _split_excess_waits(nc, limit=1):
    """walrus codegen rejects instructions carrying too many sem waits (the
    TileContext exit Drain reaches 3+). Move excess waits onto standalone
    NoOps on the same engine immediately before the instruction."""
    k = 0
    for fn in nc.m.functions:
        for bb in fn.blocks:
            insts = bb.instructions
            changed = False
            new = []
            for inst in insts:
                si = inst.sync_info
                if si is not None and si.on_wait is not None and len(si.on_wait) > limit:
                    waits = list(si.on_wait)
                    for w in waits[:-limit]:
                        nop = mybir.InstNoOp(name=f"wsplit-{k}", ins=[], outs=[])
                        k += 1
                        nop.engine = inst.engine
                        nop.sync_info = mybir.SyncInfo(on_wait=[w], on_update=[])
                        new.append(nop)
                    si.on_wait = waits[-limit:]
                    inst.sync_info = si
                    changed = True
                new.append(inst)
            if changed:
                bb.instructions = new


def _bf(v):
    return v.astype(ml_dtypes.bfloat16).astype(np.float32)


def _split3(v):
    h = _bf(v)
    r = (v - h).astype(np.float32)
    m = _bf(r)
    l = _bf((r - m).astype(np.float32))
    return h, m, l


def _ext_pair(p, g):
    """lhsT [KEXT, n] and rhs [KEXT, m] (bf16) such that
    (lhsT.T @ rhs)[n, m] ~= |p_n|^2 + |g_m|^2 - 2 p_n.g_m  at f32 precision."""
    x2 = np.einsum("nd,nd->n", p.astype(np.float64), p.astype(np.float64)).astype(
        np.float32
    )
    y2 = np.einsum("md,md->m", g.astype(np.float64), g.astype(np.float64)).astype(
        np.float32
    )
    ph, pm, pl = _split3(p)
    gh, gm, gl = _split3(g)
    x2h, x2m, x2l = _split3(x2)
    y2h, y2m, y2l = _split3(y2)
    ones_n = np.ones(p.shape[0], np.float32)
    ones_m = np.ones(g.shape[0], np.float32)

    lrows, rrows = [], []
    for k in range(3):
        for a, b in (
            (ph, gh),
            (ph, gm),
            (pm, gh),
            (ph, gl),
            (pl, gh),
            (pm, gm),
        ):
            lrows.append(-2.0 * a[:, k])
            rrows.append(b[:, k])
    for part in (x2h, x2m, x2l):
        lrows.append(part)
        rrows.append(ones_m)
    for part in (y2h, y2m, y2l):
        lrows.append(ones_n)
        rrows.append(part)
    lhsT = np.stack(lrows).astype(ml_dtypes.bfloat16)
    rhs = np.stack(rrows).astype(ml_dtypes.bfloat16)
    return lhsT, rhs


def build_program(repeat=1):
    """Single-pass kernel. repeat>1 wraps the compute body in a For_i loop
    (for timing; DMAs stay outside the loop)."""
    nc = bass.Bass()
    bf = mybir.dt.bfloat16
    f32 = mybir.dt.float32
    lA = nc.dram_tensor("lA", [KEXT, N], bf, kind="ExternalInput")
    rA = nc.dram_tensor("rA", [KEXT, M], bf, kind="ExternalInput")
    d1 = nc.dram_tensor("d1", [128, NCHUNK], f32, kind="ExternalOutput")
    d2r = nc.dram_tensor("d2r", [128, M], bf, kind="ExternalOutput")

    with tile.TileContext(nc) as tc:
        with (
            tc.tile_pool(name="inp", bufs=1) as inp,
            tc.tile_pool(name="psum", bufs=2, space="PSUM") as psum,
            tc.tile_pool(name="dstage", bufs=6) as dstage,
            tc.tile_pool(name="fold", bufs=3) as fold,
            tc.tile_pool(name="outp", bufs=1) as outp,
        ):
            tlA = inp.tile([KEXT, N], bf, tag="lA")
            nc.gpsimd.dma_start(out=tlA, in_=lA[:, :])
            trA = inp.tile([KEXT, M], bf, tag="rA")
            nc.gpsimd.dma_start(out=trA, in_=rA[:, :])

            d1_t = outp.tile([128, NCHUNK], f32, tag="d1")
            acc2 = outp.tile([128, M], bf, tag="acc2")

            def body(_i=None):
                nc.vector.memset(acc2, float("inf"))
                for c in range(NCHUNK):
                    dbfs = []
                    for h in range(2):
                        pt = psum.tile([128, HALF], f32, tag="pt")
                        for j in range(HALF // MM_N):
                            m0 = h * HALF + j * MM_N
                            nc.tensor.matmul(
                                pt[:, j * MM_N : (j + 1) * MM_N],
                                lhsT=tlA[:, c * 128 : (c + 1) * 128],
                                rhs=trA[:, m0 : m0 + MM_N],
                                start=True,
                                stop=True,
                            )
                        dbf = dstage.tile([128, HALF], bf, tag="dbf")
                        nc.scalar.copy(out=dbf, in_=pt)
                        dbfs.append(dbf)
                        hs = slice(h * HALF, (h + 1) * HALF)
                        nc.vector.tensor_tensor(
                            out=acc2[:, hs],
                            in0=dbf,
                            in1=acc2[:, hs],
                            op=mybir.AluOpType.min,
                        )
                    # row-min of the [128, 4096] chunk row via bf16 2x folds
                    f1 = fold.tile([128, HALF], bf, tag="f1")
                    nc.vector.tensor_tensor(
                        out=f1, in0=dbfs[0], in1=dbfs[1], op=mybir.AluOpType.min
                    )
                    f2 = fold.tile([128, HALF // 2], bf, tag="f2")
                    nc.vector.tensor_tensor(
                        out=f2,
                        in0=f1[:, : HALF // 2],
                        in1=f1[:, HALF // 2 :],
                        op=mybir.AluOpType.min,
                    )
                    f3 = fold.tile([128, HALF // 4], bf, tag="f3")
                    nc.vector.tensor_tensor(
                        out=f3,
                        in0=f2[:, : HALF // 4],
                        in1=f2[:, HALF // 4 :],
                        op=mybir.AluOpType.min,
                    )
                    nc.vector.tensor_reduce(
                        out=d1_t[:, c : c + 1],
                        in_=f3,
                        axis=mybir.AxisListType.X,
                        op=mybir.AluOpType.min,
                    )

            if repeat == 1:
                body()
            else:
                with tc.For_i(0, repeat, 1):
                    body()

            nc.gpsimd.dma_start(out=d1[:, :], in_=d1_t)
            nc.gpsimd.dma_start(out=d2r[:, :], in_=acc2)

    _split_excess_waits(nc)
    return nc


_PROGRAM = None


def _program():
    global _PROGRAM
    if _PROGRAM is None:
        _PROGRAM = build_program()
    return _PROGRAM


def make_in_maps(pred, gt):
    pred = np.asarray(pred, dtype=np.float32)
    gt = np.asarray(gt, dtype=np.float32)
    in_maps = []
    for b in range(B):
        la, ra = _ext_pair(pred[b], gt[b])
        in_maps.append({"lA": la, "rA": ra})
    return in_maps


def finish(results):
    """results: list of 8 dicts with d1 [128, NCHUNK] f32 and d2r [128, M]
    bf16 -> scalar loss."""
    s = 0.0
    for b in range(B):
        s += np.maximum(results[b]["d1"], 0.0).sum(dtype=np.float64)
        d2 = results[b]["d2r"].astype(np.float32).min(axis=0)
        s += np.maximum(d2, 0.0).sum(dtype=np.float64)
    return np.float32(s / (B * N))


def kernel(pred, gt):
    in_maps = make_in_maps(pred, gt)
    res = run_bass_kernel_spmd(_program(), in_maps, core_ids=list(range(B)))
    return finish(res.results)
